# revision 18
# baseline (speedup 1.0000x reference)
"""Trainium2 Bass kernel: Gaussian-splat covariance from (scaling, rotation).

Rank-2 form with normalized columns (see kernel_v3c). Three-stage software
pipeline per tile t:
  head(t):  DMA in, squares, n2, ln, exp  (scalar inv chain one tile ahead)
  body(t-1): normalized columns, products, E, weights, G outer products
  tail(t-2): M = U*G, final adds, DMA out
So the DVE never waits on the ACT ln/exp chain and the tail fills any
remaining ACT latency.
"""

import os

import numpy as np

import concourse.bass as bass
import concourse.mybir as mybir
from concourse.tile import TileContext

F16 = mybir.dt.float16
F32 = mybir.dt.float32
ALU = mybir.AluOpType
ACTF = mybir.ActivationFunctionType

SCALE_MIN = 1e-4
SCALE_MAX = 10.0
A_SC = SCALE_MAX - SCALE_MIN

N_CORES = 8
N_TOTAL = 4_000_000

F_LIST = [500, 1000, 1000, 908, 500]   # per-tile plane sizes; 128*3908*8 = 4.002M points
P_CORE = 128 * sum(F_LIST)


def _split_sync_waits(nc, nop_max=1):
    """This container's walrus encodes at most 2 sync waits per instruction
    (and none on Drain). Move excess waits onto dedicated NoOps upstream."""
    n = 0
    for bb in nc.main_func.blocks:
        out = []
        for ins in bb.instructions:
            si = ins.sync_info
            waits = list(si.on_wait) if (si is not None and si.on_wait) else []
            is_drain = type(ins).__name__ == "InstDrain"
            limit = 0 if is_drain and len(waits) > 1 else 1
            if len(waits) > limit:
                keep = waits[-limit:] if limit else []
                extra = waits[:-limit] if limit else waits
                for i0 in range(0, len(extra), nop_max):
                    n += 1
                    nop = mybir.InstNoOp(name=f"waitsplit_{n}", ins=[], outs=[])
                    nop.engine = ins.engine
                    nop.sync_info = mybir.SyncInfo(
                        on_wait=extra[i0 : i0 + nop_max], on_update=[]
                    )
                    out.append(nop)
                ins.sync_info = mybir.SyncInfo(
                    on_wait=keep, on_update=list(si.on_update or [])
                )
            out.append(ins)
        bb.instructions[:] = out
    return n


def build_nc(f_list=None):
    if f_list is None:
        f_list = F_LIST
    nc = bass.Bass()
    P = 128
    T = len(f_list)
    npts = P * sum(f_list)

    in_d = nc.declare_dram_parameter("in7", [7, npts], F16, isOutput=False)
    out_d = nc.declare_dram_parameter("out6", [6, npts], F16, isOutput=True)

    ve = nc.vector
    act = nc.scalar

    def PL(F):
        def pl(tile, i, n=1, s=1):
            r = tile[:].rearrange("p (k f) -> p k f", f=F)
            if n == 1:
                return r[:, i : i + 1]
            stop = i + (n - 1) * s + 1 if s > 0 else (i + (n - 1) * s - 1)
            if s < 0 and stop < 0:
                stop = None
            return r[:, i:stop:s]

        def bk(tile, i, n):
            r = tile[:].rearrange("p (k f) -> p k f", f=F)
            return r[:, i : i + 1].broadcast_to((P, n, F))

        return pl, bk

    bases = []
    b = 0
    for F in f_list:
        bases.append(b)
        b += P * F

    with TileContext(nc) as tc:
        with nc.allow_low_precision(reason="fp16 kernel, tol 2e-2"), \
             tc.tile_pool(name="cst", bufs=1) as cst, \
             tc.tile_pool(name="inp", bufs=2) as inp, \
             tc.tile_pool(name="outp", bufs=2) as outp, \
             tc.tile_pool(name="wk", bufs=1) as wk, \
             tc.tile_pool(name="pp", bufs=2) as pp:

            CONST = cst.tile([P, 4], F32, tag="const")

            head_st = {}
            body_st = {}

            def emit_head(t):
                F = f_list[t]
                pl, bk = PL(F)
                irow = in_d[:, bases[t] : bases[t] + P * F].rearrange(
                    "c (p f) -> p c f", p=P)
                IN = inp.tile([P, 7 * F], F16, tag="in")
                nc.sync.dma_start(
                    IN[:, : 4 * F].rearrange("p (c f) -> p c f", f=F), irow[:, 0:4])
                nc.sync.dma_start(
                    IN[:, 4 * F :].rearrange("p (c f) -> p c f", f=F), irow[:, 4:7])
                if t == 0:
                    ve.memset(CONST[:, 0:1], -1.0)
                    ve.memset(CONST[:, 1:2], float(np.log(2.0)))  # inv2 = 2/n2
                    ve.memset(CONST[:, 2:3], A_SC)
                SQ = wk.tile([P, 4 * F], F16, tag="sq")     # a d c b
                SM = wk.tile([P, 2 * F], F16, tag="sm")     # ab n2
                NRM = pp.tile([P, 2 * F], F16, tag="nrm")   # cd bd
                T1 = pp.tile([P, 2 * F], F16, tag="t1")     # ln(n2), inv2

                act.activation(SQ[:], IN[:, : 4 * F], ACTF.Square)
                ve.tensor_tensor(pl(NRM, 0, 2), pl(SQ, 2, 2), bk(SQ, 1, 2), ALU.add)
                ve.tensor_tensor(pl(SM, 0), pl(SQ, 0), pl(SQ, 3), ALU.add)
                ve.tensor_tensor(pl(SM, 1), pl(SM, 0), pl(NRM, 0), ALU.add)
                act.activation(pl(T1, 0), pl(SM, 1), ACTF.Ln)
                act.activation(pl(T1, 1), pl(T1, 0), ACTF.Exp,
                               scale=CONST[:, 0:1], bias=CONST[:, 1:2])
                head_st[t] = (IN, NRM, T1, F)

            def emit_body(t):
                IN, NRM, T1, F = head_st.pop(t)
                pl, bk = PL(F)
                NRS = wk.tile([P, 5 * F], F16, tag="nrs")   # (2r,2y,2x,2cd,2bd)/n2
                PRD = wk.tile([P, 6 * F], F16, tag="prd")
                SIG = wk.tile([P, 3 * F], F16, tag="sig")
                COL = wk.tile([P, 6 * F], F16, tag="col")
                G = pp.tile([P, 12 * F], F16, tag="g")
                UW = pp.tile([P, 2 * F], F16, tag="uw")
                W2 = pp.tile([P, 1 * F], F16, tag="w2")
                SGA = wk.tile([P, 3 * F], F16, tag="sga")

                ve.tensor_tensor(pl(NRS, 0), pl(IN, 0), pl(T1, 1), ALU.mult)
                ve.tensor_tensor(pl(NRS, 1, 2), pl(IN, 2, 2), bk(T1, 1, 2), ALU.mult)
                ve.tensor_tensor(pl(NRS, 3, 2), pl(NRM, 0, 2), bk(T1, 1, 2), ALU.mult)
                # D entries
                ve.tensor_scalar(pl(COL, 0, 2, 4), pl(NRS, 3, 2), -1.0, 1.0,
                                 ALU.mult, ALU.add)
                # products
                ve.tensor_tensor(pl(PRD, 0, 2), bk(NRS, 2, 2), pl(IN, 2, 2, -1), ALU.mult)
                ve.tensor_tensor(pl(PRD, 2), pl(NRS, 1), pl(IN, 1), ALU.mult)
                ve.tensor_tensor(pl(PRD, 3, 3), bk(NRS, 0, 3), pl(IN, 1, 3), ALU.mult)
                # E entries
                ve.tensor_tensor(pl(COL, 3, 2, -1), pl(PRD, 0, 2), pl(PRD, 3, 2), ALU.subtract)
                ve.tensor_tensor(pl(COL, 1, 2, 4), pl(PRD, 0, 2, 2), pl(PRD, 3, 2, 2), ALU.add)
                # weights
                act.activation(SIG[:], IN[:, 4 * F :], ACTF.Sigmoid)
                act.activation(SGA[:], SIG[:], ACTF.Square, scale=CONST[:, 2:3])
                ve.tensor_tensor(UW[:].rearrange("p (k f) -> p k f", f=F),
                                 pl(SGA, 0, 2), bk(SGA, 2, 2), ALU.subtract)
                act.copy(W2[:], pl(SGA, 2))
                # G
                col4 = COL[:].rearrange("p (a b f) -> p a b f", a=2, b=3)
                g4 = G[:].rearrange("p (a b f) -> p a b f", a=2, b=6)
                act.activation(g4[:, 0:1, 0:3], col4[:, 0:1], ACTF.Square)
                act.activation(g4[:, 1:2, 0:3], col4[:, 1:2], ACTF.Square)
                ve.tensor_tensor(
                    g4[:, :, 3:5],
                    col4[:, :, 0:1].broadcast_to((P, 2, 2, F)),
                    col4[:, :, 1:3], ALU.mult)
                ve.tensor_tensor(g4[:, :, 5:6], col4[:, :, 1:2], col4[:, :, 2:3], ALU.mult)
                body_st[t] = (G, UW, W2, F)

            def emit_tail(t):
                G, UW, W2, F = body_st.pop(t)
                pl, bk = PL(F)
                orow = out_d[:, bases[t] : bases[t] + P * F].rearrange(
                    "c (p f) -> p c f", p=P)
                OUT = outp.tile([P, 6 * F], F16, tag="out")
                M12 = wk.tile([P, 12 * F], F16, tag="m12")
                m4 = M12[:].rearrange("p (a b f) -> p a b f", a=2, b=6)
                g4t = G[:].rearrange("p (a b f) -> p a b f", a=2, b=6)
                uw4 = (UW[:].rearrange("p (a f) -> p a f", a=2)
                       .unsqueeze(2).broadcast_to((P, 2, 6, F)))
                ve.tensor_tensor(m4[:, 0:1], uw4[:, 0:1], g4t[:, 0:1], ALU.mult)
                ve.tensor_tensor(m4[:, 1:2], uw4[:, 1:2], g4t[:, 1:2], ALU.mult)
                ve.tensor_tensor(pl(OUT, 3, 3), pl(M12, 3, 3), pl(M12, 9, 3), ALU.add)
                nc.sync.dma_start(orow[:, 3:6], pl(OUT, 3, 3))
                ve.tensor_tensor(pl(M12, 9, 3), pl(M12, 0, 3), pl(M12, 6, 3), ALU.add)
                ve.tensor_tensor(pl(OUT, 0, 3), pl(M12, 9, 3), bk(W2, 0, 3), ALU.add)
                nc.sync.dma_start(orow[:, 0:3], pl(OUT, 0, 3))

            for t in range(T + 2):
                if t < T:
                    emit_head(t)
                if t >= 2 and t - 2 < T:
                    emit_tail(t - 2)
                if t >= 1 and t - 1 < T:
                    emit_body(t - 1)
    _split_sync_waits(nc)
    return nc


_NC_CACHE = {}


def get_nc(key="default", f_list=None):
    if key not in _NC_CACHE:
        _NC_CACHE[key] = build_nc(f_list)
    return _NC_CACHE[key]


def prep_in_maps(scaling: np.ndarray, rotation: np.ndarray):
    """Host-side: cast fp16, plane-major [7, Pc] per core (r,z,y,x,s0,s1,s2)."""
    n = scaling.shape[0]
    ntot = N_CORES * P_CORE
    in7 = np.zeros((7, ntot), dtype=np.float16)
    in7[0, :n] = rotation[:, 0]
    in7[1, :n] = rotation[:, 3]
    in7[2, :n] = rotation[:, 2]
    in7[3, :n] = rotation[:, 1]
    in7[0, n:] = 1.0
    in7[4:7, :n] = scaling.T
    return [
        {"in7": np.ascontiguousarray(in7[:, i * P_CORE : (i + 1) * P_CORE])}
        for i in range(N_CORES)
    ]


def assemble_out(results, n):
    out6 = np.concatenate([results[i]["out6"] for i in range(N_CORES)], axis=1)
    out = np.empty((n, 6), dtype=np.float32)
    out[:, 0] = out6[0, :n]
    out[:, 1] = out6[3, :n]
    out[:, 2] = out6[4, :n]
    out[:, 3] = out6[1, :n]
    out[:, 4] = out6[5, :n]
    out[:, 5] = out6[2, :n]
    return out


def kernel(scaling: np.ndarray, rotation: np.ndarray) -> np.ndarray:
    from concourse.bass_utils import run_bass_kernel_spmd

    scaling = np.asarray(scaling, dtype=np.float32)
    rotation = np.asarray(rotation, dtype=np.float32)
    n = scaling.shape[0]
    in_maps = prep_in_maps(scaling, rotation)
    nc = get_nc()
    res = run_bass_kernel_spmd(nc, in_maps, list(range(N_CORES)))
    return assemble_out(res.results, n)
